# revision 58
# baseline (speedup 1.0000x reference)
"""Multi-head attention (B=8, N=1024, D=768, 12 heads x 64) on 8 TRN2
NeuronCores, batch-parallel (one batch element per core, no collectives).

Per-core dataflow (everything transposed so no on-device transposes are
needed; x arrives host-transposed):
  - warmup matmuls on zero tiles during the input DMA keep the PE HAM
    clock-gate at 8/8 so real matmuls start at 2.4 GHz
  - qkv projection as q^T,k^T (head-dim on partitions) and v (natural)
  - q/k head dims are host-interleaved in pairs (d, d+32) -> (2d, 2d+1),
    so RoPE's rotate_half is a within-quadrant partition pair-swap:
    one DVE stream_shuffle + elementwise combine against host-precomputed
    cos / sign-folded-sin tables.  No PE rotate matmul.
  - S^T = k'.q'^T per head pair, both heads' K=64 matmuls row-tiled into
    one PE pass (concurrent via tile_position row groups); exp on ScalarE
    with the softmax scale folded into the activation's free affine
  - PV as out^T = [v|1]^T @ E^T -- the ones column yields the softmax
    denominators in psum row 64; normalization deferred past PV:
    reciprocal on DVE, partition-broadcast via a DRAM bounce for interior
    pairs (latency hidden) and via indicator-weight matmuls into PSUM for
    the final pair (keeps the critical tail on-chip)
  - projection / out-projection matmul chunks are interleaved into the
    attention kc loop so the PE has filler work while each kc's exp cooks
  - out-projection's qc=0 half runs during the qc=1 attention loop; only
    the last few matmuls trail the end.
"""
import sys

sys.path.insert(0, "/opt/trn_rl_repo")

from collections import deque

import numpy as np
import ml_dtypes

import concourse.bass as bass
import concourse.tile as tile_mod
from concourse import mybir
from concourse.alu_op_type import AluOpType
from concourse.bass_utils import run_bass_kernel_spmd
from concourse.vector_clock import ScopedClock

F32 = mybir.dt.float32
BF16 = mybir.dt.bfloat16

B, N, D = 8, 1024, 768
H, DH = 12, 64
HP = H // 2          # head pairs (two heads share a 128-partition tile)
KC = D // 128        # contraction chunks for the projections
RC = N // 128        # row chunks of the sequence
NK = N // 128        # key chunks
SCALE = DH ** -0.5
N_WARM = 20          # HAM warmup matmuls during the input DMA


# --- walrus workaround: one sync-wait per instruction ---------------------
def _patched_drain_and_barrier(self, tick_clock, wait_clock):
    drain_inst = self.nc.sync.drain()
    wait_clock.add_sem_waits(
        drain_inst.ins, ScopedClock({None: tick_clock.global_clock})
    )
    si = drain_inst.ins.sync_info
    waits = list(si.on_wait or []) if si is not None else []
    if len(waits) > 1:
        drain_inst.ins.sync_info = mybir.SyncInfo(
            on_wait=waits[:1], on_update=list(si.on_update or [])
        )
        for w in waits[1:]:
            nop = self.nc.sync.nop(nofuse=True)
            nop.ins.sync_info = mybir.SyncInfo(on_wait=[w], on_update=[])
    self.nc.all_engine_barrier()
    assert self.sems is not None
    popped = self.nc._tile_sem_poison_stack.pop()
    assert popped is self._sem_poison
    self.nc.clear_and_free_semaphores(list(self.sems.allocated().values()))
    self.nc.all_engine_barrier()


tile_mod.TileContext._drain_and_barrier = _patched_drain_and_barrier


_split_counter = [0]


def split_sync_waits(nc, max_waits=1):
    """walrus rejects instructions carrying several sem waits; spill the
    excess onto engine-matched NOPs inserted directly before the offender."""
    for f in nc.m.functions:
        for bb in f.blocks:
            il = bb.instructions
            i = 0
            while i < len(il):
                inst = il[i]
                si = inst.sync_info
                waits = list(si.on_wait or []) if si is not None else []
                if len(waits) > max_waits:
                    inst.sync_info = mybir.SyncInfo(
                        on_wait=waits[:max_waits],
                        on_update=list(si.on_update or []),
                    )
                    rest = waits[max_waits:]
                    nops = []
                    for j in range(0, len(rest), max_waits):
                        _split_counter[0] += 1
                        nop = mybir.InstNoOp(
                            name=f"I-waitsplit-{_split_counter[0]}",
                            ins=[],
                            outs=[],
                            engine=inst.engine,
                        )
                        nop.sync_info = mybir.SyncInfo(
                            on_wait=rest[j : j + max_waits], on_update=[]
                        )
                        nops.append(nop)
                    for k, nop in enumerate(nops):
                        il.insert(i + k, nop)
                    i += len(nops)
                i += 1


def _bcast_rows(dram_ap, offset_elems, parts, free):
    """AP reading dram_ap[offset : offset+free] into `parts` partitions."""
    return bass.AP(
        tensor=dram_ap.tensor,
        offset=dram_ap.offset + offset_elems,
        ap=[[0, parts], [1, free]],
    )


def build_nc(with_bias=True):
    nc = bass.Bass()
    xt_d = nc.dram_tensor("xt", [D, N], BF16, kind="ExternalInput")
    wq_d = nc.dram_tensor("wq", [D, 3 * D], BF16, kind="ExternalInput")
    wo_d = nc.dram_tensor("wo", [D, D], BF16, kind="ExternalInput")
    bo_d = nc.dram_tensor("bo", [D], BF16, kind="ExternalInput")
    cos_d = nc.dram_tensor("cos2", [128, N], BF16, kind="ExternalInput")
    sin_d = nc.dram_tensor("sin2", [128, N], BF16, kind="ExternalInput")
    sel_d = nc.dram_tensor("sel", [8, 512], BF16, kind="ExternalInput")
    out_d = nc.dram_tensor("out", [D, N], BF16, kind="ExternalOutput")

    Exp = mybir.ActivationFunctionType.Exp
    Copy = mybir.ActivationFunctionType.Copy
    SWAP_MASK = [i ^ 1 for i in range(32)]

    with tile_mod.TileContext(nc) as tc:
        with (
            tc.tile_pool(name="singles", bufs=1) as singles,
            tc.tile_pool(name="work", bufs=2) as work,
            tc.tile_pool(name="dpool", bufs=1, space="DRAM") as dpool,
        ):
            recip_d = dpool.tile([2 * H * 512], BF16)
            recip_ap = recip_d[:]

            warm_w = singles.tile([128, 128], BF16)
            nc.vector.memset(warm_w[:], 0.0)
            warm_x = singles.tile([128, 512], BF16)
            nc.vector.memset(warm_x[:], 0.0)

            # input loads: xt on SyncE's DMA rings, wv (and the rest) on
            # ScalarE's (2nd HWDGE) so the two biggest input tensors
            # transfer on disjoint ring sets in parallel
            xt_sb = singles.tile([128, KC, N], BF16)
            wv_sb = singles.tile([128, KC, D], BF16)
            for kc in range(KC):
                nc.sync.dma_start(
                    out=xt_sb[:, kc, 0:512],
                    in_=xt_d[kc * 128 : (kc + 1) * 128, 0:512],
                )
                nc.scalar.dma_start(
                    out=xt_sb[:, kc, 512:1024],
                    in_=xt_d[kc * 128 : (kc + 1) * 128, 512:1024],
                )
                nc.sync.dma_start(
                    out=wv_sb[:, kc, 0:384],
                    in_=wq_d[kc * 128 : (kc + 1) * 128, 2 * D : 2 * D + 384],
                )
                nc.scalar.dma_start(
                    out=wv_sb[:, kc, 384:768],
                    in_=wq_d[kc * 128 : (kc + 1) * 128, 2 * D + 384 : 3 * D],
                )
            # indicator weights for the final-pair reciprocal broadcast
            sel_sb = singles.tile([8, 4, 128], BF16)
            nc.scalar.dma_start(
                out=sel_sb[:], in_=sel_d[:].rearrange("p (j f) -> p j f", j=4)
            )
            cos_sb = singles.tile([128, N], BF16)
            nc.scalar.dma_start(out=cos_sb[:], in_=cos_d[:])
            sin_sb = singles.tile([128, N], BF16)
            nc.scalar.dma_start(out=sin_sb[:], in_=sin_d[:])
            bo_sb = singles.tile([1, D], BF16)
            nc.scalar.dma_start(
                out=bo_sb[:], in_=bo_d[:].rearrange("(o d) -> o d", o=1)
            )
            ones_sb = singles.tile([1, 512], BF16)
            nc.vector.memset(ones_sb[:], 1.0)
            wo_sb = singles.tile([128, KC, D], BF16)
            for c in range(KC):
                nc.scalar.dma_start(
                    out=wo_sb[:, c, :], in_=wo_d[c * 128 : (c + 1) * 128, :]
                )

            v_sb = singles.tile([128, NK, H, DH + 1], BF16)
            nc.gpsimd.memset(v_sb[:, :, :, DH : DH + 1], 1.0)
            junk_sb = singles.tile([1, 16], BF16)

            q_sb = singles.tile([128, KC, N], BF16)
            k_sb = singles.tile([128, KC, N], BF16)
            attnU_sb = singles.tile([128, KC, N], BF16)
            attn_sb = singles.tile([128, KC, N], BF16)

            # ---- v projection: v[rows, 768] = x @ Wv -------------------
            with tc.tile_pool(name="ps_v", bufs=2, space="PSUM") as ps_v:
                for i in range(N_WARM):
                    wp = ps_v.tile([128, D], F32, tag="v", name=f"warm{i}")
                    nc.tensor.matmul(
                        wp[:, 0:512], warm_w[:], warm_x[:], start=True, stop=True
                    )
                # preload the exp table set during the (ScalarE-idle)
                # v-projection phase instead of at the first real exp
                nc.scalar.activation(
                    out=junk_sb[:], in_=warm_x[0:1, 0:16], func=Exp, scale=1.0
                )
                for rc in range(RC):
                    vp = ps_v.tile([128, D], F32, tag="v")
                    for c0, w in ((0, 512), (512, 256)):
                        for kc in range(KC):
                            nc.tensor.matmul(
                                vp[:, c0 : c0 + w],
                                xt_sb[:, kc, rc * 128 : (rc + 1) * 128],
                                wv_sb[:, kc, c0 : c0 + w],
                                start=(kc == 0),
                                stop=(kc == KC - 1),
                            )
                    # strided copy into the [v | ones] per-head layout
                    nc.scalar.activation(
                        out=v_sb[:, rc, :, 0:DH],
                        in_=vp[:].rearrange("p (h d) -> p h d", h=H),
                        func=Copy,
                        scale=1.0,
                    )

            # ---- q^T / k^T projection + RoPE, split into filler chunks -
            # PE does only the 12 qkp matmuls; cast / pair-swap / rope
            # combine all run downstream on DVE.
            def proj_chunks(ps_proj, oc):
                col0 = oc * 128 if oc < KC else D + (oc - KC) * 128
                wt = work.tile(
                    [128, KC, 128], BF16, tag="wq", bufs=4, name=f"wt{oc}"
                )
                nc.sync.dma_start(
                    out=wt[:],
                    in_=wq_d[:, col0 : col0 + 128].rearrange(
                        "(c p) m -> p c m", c=KC
                    ),
                )
                q0 = work.tile([128, N], BF16, tag="q0", bufs=2, name=f"q0_{oc}")

                def half(qc2):
                    qkp = ps_proj.tile(
                        [128, 512], F32, tag="pq", name=f"qkp{oc}_{qc2}"
                    )
                    for kc in range(KC):
                        nc.tensor.matmul(
                            qkp[:],
                            wt[:, kc, :],
                            xt_sb[:, kc, qc2 * 512 : (qc2 + 1) * 512],
                            start=(kc == 0),
                            stop=(kc == KC - 1),
                        )
                    nc.vector.tensor_copy(
                        q0[:, qc2 * 512 : (qc2 + 1) * 512], qkp[:]
                    )

                def rope_tail():
                    q0s = work.tile(
                        [128, N], BF16, tag="q0s", bufs=2, name=f"q0s_{oc}"
                    )
                    nc.vector.stream_shuffle(q0s[:], q0[:], SWAP_MASK)
                    t1 = work.tile([128, N], BF16, tag="t1", bufs=2, name=f"t1_{oc}")
                    nc.vector.tensor_mul(t1[:], q0s[:], sin_sb[:])
                    t2 = work.tile([128, N], BF16, tag="t2", bufs=2, name=f"t2_{oc}")
                    nc.vector.tensor_mul(t2[:], q0[:], cos_sb[:])
                    dst = q_sb if oc < KC else k_sb
                    nc.vector.tensor_add(dst[:, oc % KC, :], t1[:], t2[:])

                return [lambda: half(0), lambda: half(1), rope_tail]

            # ---- attention: head pairs, row-tiled K=64 S^T matmuls -----
            def attn_pair(ps_att, qc, hp, filler, last=False):
                pvs = []
                for a in range(2):
                    pv = ps_att.tile(
                        [65, 512], F32, tag=f"pv{a}", bufs=1, name=f"pv{a}_{qc}_{hp}"
                    )
                    pvs.append(pv)
                for kc in range(NK):
                    st = ps_att.tile(
                        [128, N], F32, tag="st", bufs=2, name=f"st_{qc}_{hp}_{kc}"
                    )
                    for a in range(2):
                        po = 64 * a
                        nc.tensor.matmul(
                            st[:, a * 512 : (a + 1) * 512],
                            k_sb[po : po + 64, hp, kc * 128 : (kc + 1) * 128],
                            q_sb[po : po + 64, hp, qc * 512 : (qc + 1) * 512],
                            start=True,
                            stop=True,
                        )
                    e = work.tile([128, N], BF16, tag="e", bufs=2, name=f"e_{qc}_{hp}_{kc}")
                    nc.scalar.activation(out=e[:], in_=st[:], func=Exp, scale=SCALE)
                    # filler between exp and pv: PE streams useful work
                    # while this kc's exp cooks on ScalarE
                    if filler:
                        filler.popleft()()
                    for a in range(2):
                        nc.tensor.matmul(
                            pvs[a][:],
                            v_sb[:, kc, 2 * hp + a, :],
                            e[:, a * 512 : (a + 1) * 512],
                            start=(kc == 0),
                            stop=(kc == NK - 1),
                        )
                # final pair: evacuations split across DVE and the (now
                # idle) ScalarE; the denominator-row DMAs go FIRST (they
                # gate the reciprocal) and split across both HWDGE rings
                pvts = []
                sums_t = work.tile([8, 128], BF16, tag="sums", bufs=2, name=f"sums{qc}_{hp}")
                for a in range(2):
                    pvt = work.tile(
                        [65, 512], BF16, tag="pvt", bufs=3, name=f"pvt{qc}_{2*hp+a}"
                    )
                    if last and a == 1:
                        nc.scalar.activation(
                            out=pvt[:], in_=pvs[a][:], func=Copy, scale=1.0
                        )
                    else:
                        nc.vector.tensor_copy(pvt[:], pvs[a][:])
                    if last:
                        eng_s = nc.scalar if a == 0 else nc.sync
                        eng_s.dma_start(
                            out=sums_t[a * 4 : a * 4 + 4, :], in_=pvt[64:65, :]
                        )
                        eng_a = nc.sync if a == 0 else nc.scalar
                        eng_a.dma_start(
                            out=attnU_sb[
                                64 * a : 64 * a + 64, hp, qc * 512 : (qc + 1) * 512
                            ],
                            in_=pvt[0:64, :],
                        )
                    else:
                        nc.sync.dma_start(
                            out=attnU_sb[
                                64 * a : 64 * a + 64, hp, qc * 512 : (qc + 1) * 512
                            ],
                            in_=pvt[0:64, :],
                        )
                        nc.sync.dma_start(
                            out=sums_t[a * 4 : a * 4 + 4, :], in_=pvt[64:65, :]
                        )
                    pvts.append(pvt)
                return sums_t

            # ---- softmax normalization for one (head pair, qc) --------
            # interior pairs broadcast the reciprocal rows via a DRAM
            # bounce (latency hidden under later work); the final pair
            # broadcasts the raw denominator rows via K=1 matmuls into
            # PSUM and divides -- no reciprocal, no DMA on the tail
            def norm(hp, qc, sums_t, ps_last=None):
                rcp = work.tile([8, 128], BF16, tag="rcp", bufs=2, name=f"rcp{hp}_{qc}")
                with nc.allow_low_precision(
                    reason="bf16 softmax denominators; rel-err budget is 2e-2"
                ):
                    nc.vector.reciprocal(rcp[:], sums_t[:])
                if ps_last is not None:
                    rbp = ps_last.tile([128, 512], F32, tag="pq", name=f"rbp{hp}_{qc}")
                    for j in range(4):
                        nc.tensor.matmul(
                            rbp[:, j * 128 : (j + 1) * 128],
                            sel_sb[:, j, :],
                            rcp[:],
                            start=(j == 0),
                            stop=(j == 3),
                        )
                    nc.vector.tensor_mul(
                        attn_sb[:, hp, qc * 512 : (qc + 1) * 512],
                        attnU_sb[:, hp, qc * 512 : (qc + 1) * 512],
                        rbp[:],
                    )
                    return
                o0 = (qc * H + 2 * hp) * 512
                nc.sync.dma_start(
                    out=recip_d[o0 : o0 + 1024].rearrange("(p f) -> p f", p=8),
                    in_=rcp[:],
                )
                rb = work.tile([128, 512], BF16, tag="rb", bufs=3, name=f"rb{hp}_{qc}")
                nc.sync.dma_start(
                    out=rb[0:64, :], in_=_bcast_rows(recip_ap, o0, 64, 512)
                )
                nc.sync.dma_start(
                    out=rb[64:128, :], in_=_bcast_rows(recip_ap, o0 + 512, 64, 512)
                )
                nc.vector.tensor_mul(
                    attn_sb[:, hp, qc * 512 : (qc + 1) * 512],
                    attnU_sb[:, hp, qc * 512 : (qc + 1) * 512],
                    rb[:],
                )

            # ---- out-projection for one (128-col tile, qc) -------------
            def outproj_chunks(
                ps_fin, oc, qc, tail_cast_act=False, split_last=False, split_store=False
            ):
                fp = ps_fin.tile([128, 512], F32, tag="pq", name=f"fin{oc}_{qc}")

                def crange(cs):
                    for c in cs:
                        nc.tensor.matmul(
                            fp[:],
                            wo_sb[:, c, oc * 128 : (oc + 1) * 128],
                            attn_sb[:, c, qc * 512 : (qc + 1) * 512],
                            start=(c == 0),
                            stop=(not with_bias and c == KC - 1),
                        )
                    if cs[-1] != KC - 1:
                        return
                    if with_bias:
                        nc.tensor.matmul(
                            fp[:],
                            bo_sb[0:1, oc * 128 : (oc + 1) * 128],
                            ones_sb[:],
                            start=False,
                            stop=True,
                        )
                    fsb = work.tile(
                        [128, 512], BF16, tag="fsb", bufs=3, name=f"fsb{oc}_{qc}"
                    )
                    if tail_cast_act:
                        nc.scalar.activation(
                            out=fsb[:], in_=fp[:], func=Copy, scale=1.0
                        )
                    else:
                        nc.vector.tensor_copy(fsb[:], fp[:])
                    if split_store:
                        # tail ocs: split the store across both HWDGE
                        # rings so the final transfer latency halves
                        nc.sync.dma_start(
                            out=out_d[
                                oc * 128 : (oc + 1) * 128,
                                qc * 512 : qc * 512 + 256,
                            ],
                            in_=fsb[:, 0:256],
                        )
                        nc.scalar.dma_start(
                            out=out_d[
                                oc * 128 : (oc + 1) * 128,
                                qc * 512 + 256 : (qc + 1) * 512,
                            ],
                            in_=fsb[:, 256:512],
                        )
                    else:
                        nc.sync.dma_start(
                            out=out_d[
                                oc * 128 : (oc + 1) * 128,
                                qc * 512 : (qc + 1) * 512,
                            ],
                            in_=fsb[:],
                        )

                if split_last:
                    return [
                        lambda: crange((0, 1)),
                        lambda: crange((2, 3)),
                        lambda: crange((4,)),
                        lambda: crange((5,)),
                    ]
                return [
                    lambda: crange((0, 1)),
                    lambda: crange((2, 3)),
                    lambda: crange((4, 5)),
                ]

            with (
                tc.tile_pool(name="ps_att", bufs=1, space="PSUM") as ps_att,
                tc.tile_pool(name="ps_x", bufs=2, space="PSUM") as ps_x,
            ):
                pend = deque()
                for f in proj_chunks(ps_x, 0) + proj_chunks(ps_x, KC):
                    f()
                for hp in range(HP):
                    if hp + 1 < HP:
                        pend.extend(proj_chunks(ps_x, hp + 1))
                        pend.extend(proj_chunks(ps_x, KC + hp + 1))
                    s_t = attn_pair(ps_att, 0, hp, pend)
                    while pend:
                        pend.popleft()()
                    norm(hp, 0, s_t)
                oc0_tail = None
                for hp in range(HP):
                    pend.extend(outproj_chunks(ps_x, hp, 0))
                    last = hp == HP - 1
                    if last:
                        # first final-oc's early accumulations ride the
                        # last pair's filler slots; its c=4/c=5 chunks run
                        # around the final norm chain below
                        oc0_tail = outproj_chunks(
                            ps_x, 0, 1, split_last=True, split_store=True
                        )
                        pend.extend(oc0_tail[:2])
                    s_t = attn_pair(ps_att, 1, hp, pend, last=last)
                    while pend:
                        pend.popleft()()
                    if not last:
                        norm(hp, 1, s_t)
                oc0_tail[2]()                    # c=4 fills PE while recip runs
                norm(HP - 1, 1, s_t, ps_last=ps_x)
                oc0_tail[3]()                    # c=5 + store
                for oc in range(1, KC):
                    for f in outproj_chunks(
                        ps_x, oc, 1, tail_cast_act=(oc % 2 == 1), split_store=True
                    ):
                        f()

    split_sync_waits(nc, max_waits=1)
    return nc


def _host_prep(x, w_qkv, w_out, b_out):
    bf = ml_dtypes.bfloat16
    inv_freq = 1.0 / (10000.0 ** (np.arange(0, DH, 2, dtype=np.float32) / DH))
    t = np.arange(N, dtype=np.float32)
    freqs = np.outer(t, inv_freq)
    emb = np.concatenate([freqs, freqs], axis=1)        # [N, DH]
    cos_t = np.cos(emb).T.astype(np.float32)            # [DH, N]
    sin_t = np.sin(emb).T.astype(np.float32)

    # interleave head dims in pairs (d, d+32) -> rows (2d, 2d+1) so
    # rotate_half becomes a partition pair-swap; fold rotate's sign into
    # the sin table (row 2d carries -sin)
    perm64 = np.ravel(
        np.stack([np.arange(32), np.arange(32) + 32], axis=1)
    )                                                   # [0,32,1,33,...]
    signs = np.tile(np.array([-1.0, 1.0], np.float32), 32)[:, None]
    cos_p = cos_t[perm64]
    sin_p = sin_t[perm64] * signs
    cos2 = np.tile(cos_p, (2, 1)).astype(bf)
    sin2 = np.tile(sin_p, (2, 1)).astype(bf)

    inner = H * DH
    qk_perm = np.concatenate([h * DH + perm64 for h in range(H)])
    wq = np.asarray(w_qkv, dtype=np.float32).copy()
    wq[:, 0:inner] = wq[:, 0:inner][:, qk_perm]
    wq[:, inner : 2 * inner] = wq[:, inner : 2 * inner][:, qk_perm]

    # indicator weights for the final-pair reciprocal broadcast:
    # sel[h*4+j, j, h*64:(h+1)*64] = 1
    sel = np.zeros((8, 4, 128), np.float32)
    for h in range(2):
        for j in range(4):
            sel[h * 4 + j, j, h * 64 : (h + 1) * 64] = 1.0

    xt = np.ascontiguousarray(x.transpose(0, 2, 1)).astype(bf)
    shared = {
        "wq": np.ascontiguousarray(wq).astype(bf),
        "wo": np.ascontiguousarray(w_out).astype(bf),
        "bo": np.ascontiguousarray(b_out).astype(bf),
        "cos2": np.ascontiguousarray(cos2),
        "sin2": np.ascontiguousarray(sin2),
        "sel": np.ascontiguousarray(sel.reshape(8, 512)).astype(bf),
    }
    return [dict(shared, xt=np.ascontiguousarray(xt[i])) for i in range(B)]


_NC_CACHE = {}
LAST_EXEC_NS = [None]


def _run(in_maps, trace=False, with_bias=True):
    if with_bias not in _NC_CACHE:
        _NC_CACHE[with_bias] = build_nc(with_bias=with_bias)
    res = run_bass_kernel_spmd(
        _NC_CACHE[with_bias], in_maps, list(range(B)), trace=trace
    )
    LAST_EXEC_NS[0] = res.exec_time_ns
    out_t = np.stack(
        [np.asarray(res.results[i]["out"]).astype(np.float32) for i in range(B)]
    )
    return np.ascontiguousarray(out_t.transpose(0, 2, 1))


def kernel(x, w_qkv, w_out, b_out, _trace=False):
    b_out = np.asarray(b_out, dtype=np.float32)
    in_maps = _host_prep(
        np.asarray(x, dtype=np.float32),
        np.asarray(w_qkv, dtype=np.float32),
        np.asarray(w_out, dtype=np.float32),
        b_out,
    )
    return _run(in_maps, trace=_trace, with_bias=bool(np.any(b_out)))
